# revision 1
# baseline (speedup 1.0000x reference)
"""AdaConv (nn_AdaConv_46445776339355) — 8-core TRN2 Bass kernel, v2.

Strategy
--------
Data-parallel over batch N=8: core n owns sample n end-to-end for the heavy
instance-norm + grouped-conv work.  The kernel *generator* (dw_w is 256 MiB)
is tensor-parallel: core j holds the output-channel shard j of dw_w / pwk_w,
computes the generated kernels for ALL samples on its shard, and an AllToAll
routes each sample's kernels to its owning core.

Algebraic fusions (all computed on device):
  * pointwise o depthwise = one fused per-group kernel  F_t = P @ W_t
  * instance norm folded into the fused kernels:
        y = sum_t F_t @ ((x-mu)/sigma)_pad = sum_t (F_t/sigma_ci) @ x_pad - B
    with B = sum_t (F_t/sigma) @ mu  (position independent, reflect-pad safe)
  * biases (dw_b, pwk_b, pwb_b) folded in via K=1 matmul rows.

The grouped conv (8 groups of 64->64 ch, 3x3) runs as 4 concurrent 64x64
matmuls in the 4 PE-array quadrants (tile_position packing), bf16, 9
shifted-AP taps accumulating in PSUM.

v2 scheduling (vs v1):
  * weight stream spread over 3 DMA queues (sync/scalar/vector) with
    partition-major host packing; images on the gpsimd queue (biggest
    descriptors, best observed queue BW); img p3 on the vector queue.
  * stats split: sum-of-squares on ScalarE (Square+accum), sums on VectorE,
    both overlapping the weight stream.
  * AllToAll issued as soon as the generators drain.
  * F build: P loads hoisted (v1 reloaded them 9x), taps packed into PSUM
    tiles; B bias via tap-summed F (4 matmuls instead of 72).
  * output staged + written in bf16 (halves output DMA), host casts back.
  * F/bias for half 1 emitted 2 blocks into conv h0 (PE never idles between
    halves); pair-B evacs alternate VectorE/GpSimd.
"""

import sys
import numpy as np

sys.path.insert(0, "/opt/trn_rl_repo")

import ml_dtypes

BF16 = ml_dtypes.bfloat16

# ---------------- problem constants (hardcoded per the harness contract) ----
N = 8            # batch == number of cores
C = 512          # channels
H = W = 128
HW = H * W       # 16384
PW = W + 2       # 130 padded
PA = PW * PW     # 16900
SD = 512         # style dim
NG = 8           # groups
GS = 64          # group size (channels per group)
KDW = SD * 4     # 2048 contraction dim of the dw generator
OSH = 4096       # dw/pwk output-channel shard per core (32768/8)
NTAP = 9
EPS = 1e-5
VAR_CORR = float(HW) / float(HW - 1)  # ddof=1 correction

NKT = KDW // 128          # 16 contraction tiles for the dw generator
DW_CH = 2                 # kt per dwt DMA chunk
N_DWCH = NKT // DW_CH     # 8 chunks

# device output channel order: per pair of groups (2h, 2h+1) natural, the odd
# pairs (pB) have their two 64-blocks swapped (quadrant output packing).
TAU_BLOCK = [0, 1, 3, 2, 4, 5, 7, 6]  # true 64-block of device 64-block d


def _host_prep(style_encoding, predicted, dw_w, dw_b, pwk_w, pwk_b, pwb_w, pwb_b):
    """Pure data-movement / dtype-cast host prep. Returns per-core input maps."""
    f32 = np.float32
    se = np.asarray(style_encoding, f32)
    pred = np.asarray(predicted, f32)

    # --- patches for the dw generator conv: reflect pad 1, 2x2 windows s=2 ---
    sep = np.pad(se, ((0, 0), (0, 0), (1, 1), (1, 1)), mode="reflect")  # (8,512,6,6)
    blocks = sep.reshape(N, SD, 3, 2, 3, 2)  # [n,c,oy,ky,ox,kx]
    patches = np.ascontiguousarray(
        blocks.transpose(1, 3, 5, 0, 2, 4).reshape(KDW, N * NTAP)
    ).astype(BF16)  # [(c,ky,kx), (n,oy,ox)] = [2048, 72]
    patches_t = np.ascontiguousarray(
        patches.reshape(16, 128, N * NTAP).transpose(1, 0, 2).reshape(128, 16 * N * NTAP)
    )

    # --- dw generator weights, transposed + sharded on output channels,
    #     partition-major packed: dwt_pk[p, kt*OSH + m] = dwt[kt*128+p, m]
    dwt_full = np.ascontiguousarray(dw_w.reshape(C * GS, KDW).T).astype(BF16)  # [2048, 32768]
    dwb_full = np.asarray(dw_b, f32).reshape(1, C * GS).astype(BF16)

    # --- pwk: permute columns to (g, cm, co2) so the gathered row IS P^T ---
    pwk_t = np.asarray(pwk_w, f32).reshape(NG, GS, GS, SD)  # [g, co2, cm, sd]
    pwkt_full = np.ascontiguousarray(
        pwk_t.transpose(3, 0, 2, 1).reshape(SD, C * GS)
    ).astype(BF16)  # [sd, (g, cm, co2)]
    pwkb_full = (
        np.asarray(pwk_b, f32).reshape(NG, GS, GS).transpose(0, 2, 1).reshape(1, C * GS)
    ).astype(BF16)

    # --- pwb: transposed, columns in DEVICE channel order tau ---
    tau_rows = np.concatenate([np.arange(GS) + t * GS for t in TAU_BLOCK])  # [512]
    pwbt = np.ascontiguousarray(np.asarray(pwb_w, f32)[tau_rows, :].T).astype(BF16)  # [sd, out_dev]
    pwbt_t = np.ascontiguousarray(
        pwbt.reshape(4, 128, C).transpose(1, 0, 2).reshape(128, 4 * C))
    pwbb = np.asarray(pwb_b, f32)[tau_rows].reshape(1, C).astype(BF16)

    # --- styleT for sd computation on device: pre-tiled [128, kt, (n, px)] f32 ---
    styleT = np.ascontiguousarray(se.transpose(1, 0, 2, 3).reshape(SD, N * 16)).astype(f32)
    styleT_t = np.ascontiguousarray(
        styleT.reshape(4, 128, N * 16).transpose(1, 0, 2).reshape(128, 4 * N * 16))

    in_maps = []
    for j in range(N):
        pp = np.pad(pred[j], ((0, 0), (1, 1), (1, 1)), mode="reflect").reshape(C, PA)
        sel = np.zeros((128, N), f32)
        sel[:, j] = 1.0
        shard = dwt_full[:, j * OSH:(j + 1) * OSH]  # [2048, 4096]
        dwt_pk = shard.reshape(NKT, 128, OSH).transpose(1, 0, 2)  # [128, 16, 4096]
        pwk_shard = pwkt_full[:, j * OSH:(j + 1) * OSH]  # [512, 4096]
        pwk_pk = pwk_shard.reshape(4, 128, OSH).transpose(1, 0, 2)  # [128, 4, 4096]
        # unified weight stream: dw kts 0-15 then pwk kts 0-3 -> [128, 20*4096]
        w_all = np.ascontiguousarray(
            np.concatenate([dwt_pk, pwk_pk], axis=1).reshape(128, 20 * OSH)
        )
        in_maps.append(
            dict(
                pred_pad=np.ascontiguousarray(pp).astype(BF16),
                patches=patches_t,
                styleT=styleT_t,
                sel=sel,
                dwt=w_all,
                dwb=np.ascontiguousarray(dwb_full[:, j * OSH:(j + 1) * OSH]),
                pwkb=np.ascontiguousarray(pwkb_full[:, j * OSH:(j + 1) * OSH]),
                pwbt=pwbt_t,
                pwbb=pwbb,
            )
        )
    return in_maps


def _unshard(results):
    """results[j]['out'] is [512, 16384] bf16 in device channel order."""
    out = np.empty((N, C, H, W), np.float32)
    for j in range(N):
        dev = np.asarray(results[j]["out"]).astype(np.float32).reshape(C, H, W)
        for d, t in enumerate(TAU_BLOCK):
            out[j, t * GS:(t + 1) * GS] = dev[d * GS:(d + 1) * GS]
    return out


# how many 512-px N-tiles are accumulated per psum set before evacuation
CONV_NT = 2


def build_nc():
    from concourse import bacc, mybir, tile
    from contextlib import ExitStack

    dt = mybir.dt
    AF = mybir.ActivationFunctionType
    ALU = mybir.AluOpType

    nc = bacc.Bacc(num_devices=N)

    pred_pad = nc.declare_dram_parameter("pred_pad", [C, PA], dt.bfloat16, isOutput=False)
    patches = nc.declare_dram_parameter("patches", [128, 16 * N * NTAP], dt.bfloat16, isOutput=False)
    styleT = nc.declare_dram_parameter("styleT", [128, 4 * N * 16], dt.float32, isOutput=False)
    sel = nc.declare_dram_parameter("sel", [128, N], dt.float32, isOutput=False)
    dwt = nc.declare_dram_parameter("dwt", [128, 20 * OSH], dt.bfloat16, isOutput=False)
    dwb = nc.declare_dram_parameter("dwb", [1, OSH], dt.bfloat16, isOutput=False)
    pwkb = nc.declare_dram_parameter("pwkb", [1, OSH], dt.bfloat16, isOutput=False)
    pwbt = nc.declare_dram_parameter("pwbt", [128, 4 * C], dt.bfloat16, isOutput=False)
    pwbb = nc.declare_dram_parameter("pwbb", [1, C], dt.bfloat16, isOutput=False)
    out_dev = nc.declare_dram_parameter("out", [C, HW], dt.bfloat16, isOutput=True)

    replica = [list(range(N))]

    with tile.TileContext(nc) as tc, ExitStack() as ctx:
        dram = ctx.enter_context(tc.tile_pool(name="dram", bufs=1, space="DRAM"))
        gen_all = dram.tile([N, 10, OSH], dt.bfloat16, tag="gall")
        gen_out = dram.tile([N, 10, OSH], dt.bfloat16, tag="gout")

        const_p = ctx.enter_context(tc.tile_pool(name="const", bufs=1))
        pred_p = ctx.enter_context(tc.tile_pool(name="pred", bufs=1))
        stats_p = ctx.enter_context(tc.tile_pool(name="stats", bufs=2))
        fker_p = ctx.enter_context(tc.tile_pool(name="fker", bufs=1))
        # weight-stream + gen-staging pools are scoped: they close after the
        # generator phase, freeing SBUF for the F/evac pools below
        wstream_ctx = tc.tile_pool(name="wstream", bufs=3)
        wstream_p = wstream_ctx.__enter__()
        gstg_ctx = tc.tile_pool(name="gstg", bufs=2)
        gstg_p = gstg_ctx.__enter__()

        # ------------------------------------------------ tiny consts (sync q)
        pt = const_p.tile([128, 16, N * NTAP], dt.bfloat16, tag="pt")
        nc.sync.dma_start(out=pt[:], in_=patches.rearrange("p (kt m) -> p kt m", kt=16))
        st = const_p.tile([128, 4, N * 16], dt.float32, tag="st")
        nc.sync.dma_start(out=st[:], in_=styleT.rearrange("p (kt m) -> p kt m", kt=4))
        sel_sb = const_p.tile([128, N], dt.float32, tag="sel")
        nc.sync.dma_start(out=sel_sb[:], in_=sel[:, :])

        # ------------------------------------------------ big input streams
        # Only SP (sync), Activation (scalar) and Pool (gpsimd) can DMA.
        # Unified weight stream (dw kts 0-15 + pwk kts 16-19) as 10 chunks
        # [128, 2, 4096] (16 KiB descriptors).  The pwk chunks load FIRST
        # (pwk generation runs before dw generation so its PSUM banks free
        # early and the chunk-slot rotation has no cycle): sync gets pwk0,
        # c0, c2, c4, c6; scalar gets pwk1, c1, c3, c5, c7.  All four image
        # chunks on gpsimd (33.8 KiB descriptors).
        w_v = dwt.rearrange("p (kt m) -> p kt m", kt=20)
        pk_ch = []
        for c in (8, 9):
            wtile = wstream_p.tile([128, DW_CH, OSH], dt.bfloat16, name=f"pk{c}",
                                   tag="w")
            nc.gpsimd.dma_start(
                out=wtile[:], in_=w_v[:, c * DW_CH:(c + 1) * DW_CH, :])
            pk_ch.append(wtile)
        # pwk-bias rows: tiny loads ahead of the weight chunks on gpsimd so
        # the pwk bias matmuls (which free the pwk PSUM banks) never stall
        pkbias_t = []
        for b in range(8):
            bt = gstg_p.tile([1, 512], dt.bfloat16, name="bt", tag="bias")
            nc.gpsimd.dma_start(out=bt[:], in_=pwkb[0:1, b * 512:(b + 1) * 512])
            pkbias_t.append(bt)
        w_ch = []
        ch_eng = {0: nc.sync, 1: nc.scalar, 2: nc.gpsimd, 3: nc.sync,
                  4: nc.scalar, 5: nc.gpsimd, 6: nc.sync, 7: nc.scalar}
        for c in range(8):
            wtile = wstream_p.tile([128, DW_CH, OSH], dt.bfloat16, name=f"wt{c}",
                                   tag="w")
            ch_eng[c].dma_start(
                out=wtile[:], in_=w_v[:, c * DW_CH:(c + 1) * DW_CH, :])
            w_ch.append(wtile)

        img = [pred_p.tile([128, PA], dt.bfloat16, name=f"img{p}", tag=f"img{p}")
               for p in range(4)]

        ones = const_p.tile([1, 128], dt.bfloat16, tag="ones")
        nc.vector.memset(ones[:], 1.0)

        # ------------------------------------------------ sd = mean(style, px)
        sdf = const_p.tile([128, 4, N], dt.float32, tag="sdf")
        sdb = const_p.tile([128, 4, N], dt.bfloat16, tag="sdb")
        sdnb = const_p.tile([128, 4], dt.bfloat16, tag="sdnb")  # own-sample column
        tmp8 = stats_p.tile([128, N], dt.float32, tag="tmp8")
        sdn_f = const_p.tile([128, 4], dt.float32, tag="sdnf")
        for kt in range(4):
            nc.vector.tensor_reduce(
                out=sdf[:, kt, :],
                in_=st[:, kt, :].rearrange("p (n x) -> p n x", x=16),
                axis=mybir.AxisListType.X,
                op=ALU.add,
            )
            nc.vector.tensor_scalar(
                out=sdb[:, kt, :], in0=sdf[:, kt, :], scalar1=1.0 / 16.0,
                scalar2=None, op0=ALU.mult,
            )
            nc.vector.tensor_tensor(
                out=tmp8[:], in0=sdf[:, kt, :], in1=sel_sb[:], op=ALU.mult
            )
            nc.vector.tensor_reduce(
                out=sdn_f[:, kt:kt + 1], in_=tmp8[:], axis=mybir.AxisListType.X, op=ALU.add
            )
        nc.vector.tensor_scalar(
            out=sdnb[:], in0=sdn_f[:], scalar1=1.0 / 16.0, scalar2=None, op0=ALU.mult
        )

        # ------------------------------------------------ instance-norm stats
        rstd_sb = const_p.tile([128, 4], dt.float32, tag="rstd")
        muneg_sb = const_p.tile([128, 4], dt.bfloat16, tag="muneg")
        sq_dummy = stats_p.tile([128, 8 * 128], dt.float8e4, tag="sqdmy", bufs=1)

        def stats_sq(p):
            """ScalarE: sacc[:, j] = sum over chunk j of x^2 (Square+accum)."""
            view = img[p].rearrange("p (r c) -> p r c", c=PW)
            sacc = stats_p.tile([128, 16], dt.float32, name=f"sacc{p}", tag=f"sacc{p}")
            dview = sq_dummy.rearrange("p (r c) -> p r c", c=128)
            for j in range(16):
                xs = view[:, 1 + 8 * j:1 + 8 * (j + 1), 1:129]
                nc.scalar.activation(
                    out=dview[:], in_=xs, func=AF.Square,
                    accum_out=sacc[:, j:j + 1],
                )
            return sacc

        def stats_sum(p, sacc):
            """VectorE: full-image sum + combines -> muneg, var+eps."""
            view = img[p].rearrange("p (r c) -> p r c", c=PW)
            ssum = stats_p.tile([128, 1], dt.float32, name="ssum", tag="ssum")
            nc.vector.tensor_reduce(
                out=ssum[:], in_=view[:, 1:129, 1:129],
                axis=mybir.AxisListType.XY, op=ALU.add
            )
            ssq = stats_p.tile([128, 1], dt.float32, name="ssq", tag="ssq")
            nc.vector.tensor_reduce(
                out=ssq[:], in_=sacc[:], axis=mybir.AxisListType.X, op=ALU.add
            )
            mu = stats_p.tile([128, 1], dt.float32, name="mu", tag="mu")
            nc.vector.tensor_scalar(
                out=mu[:], in0=ssum[:], scalar1=1.0 / HW, scalar2=None, op0=ALU.mult
            )
            nc.vector.tensor_scalar(
                out=muneg_sb[:, p:p + 1], in0=mu[:], scalar1=-1.0,
                scalar2=None, op0=ALU.mult,
            )
            ex2 = stats_p.tile([128, 1], dt.float32, name="ex2", tag="ex2")
            nc.vector.tensor_scalar(
                out=ex2[:], in0=ssq[:], scalar1=1.0 / HW, scalar2=None, op0=ALU.mult
            )
            mu2 = stats_p.tile([128, 1], dt.float32, name="mu2", tag="mu2")
            nc.vector.tensor_tensor(out=mu2[:], in0=mu[:], in1=mu[:], op=ALU.mult)
            varp = stats_p.tile([128, 1], dt.float32, name="varp", tag="varp")
            nc.vector.tensor_tensor(out=varp[:], in0=ex2[:], in1=mu2[:], op=ALU.subtract)
            vtmp = stats_p.tile([128, 1], dt.float32, name="vtmp", tag=f"sm2_{p}")
            nc.vector.tensor_scalar(
                out=vtmp[:], in0=varp[:], scalar1=VAR_CORR, scalar2=EPS,
                op0=ALU.mult, op1=ALU.add,
            )
            return vtmp

        def stats_finish(p, vtmp):
            stdt = stats_p.tile([128, 1], dt.float32, name="stdt", tag=f"sm3_{p}")
            nc.scalar.sqrt(stdt[:], vtmp[:])
            nc.vector.reciprocal(out=rstd_sb[:, p:p + 1], in_=stdt[:])

        # ------------------------------------------------ generator phase (PE)
        with tc.tile_pool(name="psgen", bufs=8, space="PSUM") as psum_g:
            # DRAM-write staging for the generated kernels: batched copies
            # into one [72, 4096] tile -> two 8 KiB-descriptor DMA writes
            gs4 = gstg_p.tile([N * NTAP, 8 * 512], dt.bfloat16, tag="gs4", bufs=1)

            # ---- pwk generator FIRST (frees its PSUM banks early; its
            # chunks head the weight stream so no slot-rotation cycle)
            ps_pk = [psum_g.tile([128, 512], dt.float32, name=f"pkg{b}", tag="g")
                     for b in range(8)]
            for kt in range(4):
                pk = pk_ch[kt // 2]
                for b in range(8):
                    nc.tensor.matmul(
                        out=ps_pk[b][:N, :],
                        lhsT=sdb[:, kt, :],
                        rhs=pk[:, kt % 2, b * 512:(b + 1) * 512],
                        start=(kt == 0), stop=False,
                    )
            for b in range(8):
                nc.tensor.matmul(
                    out=ps_pk[b][:N, :],
                    lhsT=ones[:1, :N],
                    rhs=pkbias_t[b][:1, :],
                    start=False, stop=True,
                )
                nc.vector.tensor_scalar(out=gs4[0:N, b * 512:(b + 1) * 512],
                                        in0=ps_pk[b][:N, :],
                                        scalar1=1.0, scalar2=None, op0=ALU.mult)
            nc.sync.dma_start(out=gen_all[:, NTAP, :], in_=gs4[0:N, :])

            # dw-bias rows next on gpsimd (ahead of the big image transfers)
            dwbias_t = []
            for b in range(8):
                bt = gstg_p.tile([1, 512], dt.bfloat16, name="bt", tag="bias")
                nc.gpsimd.dma_start(out=bt[:], in_=dwb[0:1, b * 512:(b + 1) * 512])
                dwbias_t.append(bt)

            # ---- images: AFTER the weight stream on each queue, so weights
            # get the full aggregate DMA bandwidth first
            nc.gpsimd.dma_start(out=img[0][:], in_=pred_pad[0:128, :])
            nc.gpsimd.dma_start(out=img[1][:], in_=pred_pad[128:256, :])
            nc.sync.dma_start(out=img[2][:], in_=pred_pad[256:384, :])
            nc.scalar.dma_start(out=img[3][:], in_=pred_pad[384:512, :])
            # stats for pairs 0,1 (ScalarE squares + VectorE sums)
            sacc0 = stats_sq(0)
            sacc1 = stats_sq(1)
            vt0 = stats_sum(0, sacc0)
            stats_finish(0, vt0)
            vt1 = stats_sum(1, sacc1)
            stats_finish(1, vt1)

            # ---- dw generator
            ps_dw = [psum_g.tile([128, 512], dt.float32, name=f"dwg{b}", tag="g")
                     for b in range(8)]
            for kt in range(NKT):
                wt = w_ch[kt // DW_CH]
                for b in range(8):
                    nc.tensor.matmul(
                        out=ps_dw[b][:N * NTAP, :],
                        lhsT=pt[:, kt, :],
                        rhs=wt[:, kt % DW_CH, b * 512:(b + 1) * 512],
                        start=(kt == 0), stop=False,
                    )
            for b in range(8):
                nc.tensor.matmul(
                    out=ps_dw[b][:N * NTAP, :],
                    lhsT=ones[:1, :N * NTAP],
                    rhs=dwbias_t[b][:1, :],
                    start=False, stop=True,
                )
                nc.vector.tensor_scalar(out=gs4[:, b * 512:(b + 1) * 512],
                                        in0=ps_dw[b][:N * NTAP, :],
                                        scalar1=1.0, scalar2=None, op0=ALU.mult)
            nc.sync.dma_start(out=gen_all[:, 0:NTAP, :], in_=gs4[:, :])

            # AllToAll ASAP (gpsimd queue)
            nc.gpsimd.collective_compute(
                "AllToAll",
                ALU.bypass,
                replica_groups=replica,
                ins=[gen_all[:, :, :].opt()],
                outs=[gen_out[:, :, :].opt()],
            )

        # weight stream fully consumed — free its SBUF for the F/evac pools
        gstg_ctx.__exit__(None, None, None)
        wstream_ctx.__exit__(None, None, None)
        fload_p = ctx.enter_context(tc.tile_pool(name="fload", bufs=3))
        stage_p = ctx.enter_context(tc.tile_pool(name="stage", bufs=3))

        pwbt_sb = fload_p.tile([128, 4, C], dt.bfloat16, tag="pwbt", bufs=1)
        nc.sync.dma_start(out=pwbt_sb[:], in_=pwbt.rearrange("p (kt m) -> p kt m", kt=4))
        pwbb_sb = fload_p.tile([1, C], dt.bfloat16, tag="pwbb", bufs=1)
        nc.sync.dma_start(out=pwbb_sb[:], in_=pwbb[:, :])
        pwb_sb = const_p.tile([128, 4], dt.float32, tag="pwbv")

        # stats for pairs 2,3: Square passes on ScalarE, sums on VectorE —
        # both run while the AllToAll is in flight
        sacc2 = stats_sq(2)
        sacc3 = stats_sq(3)
        vt2 = stats_sum(2, sacc2)
        stats_finish(2, vt2)
        vt3 = stats_sum(3, sacc3)
        stats_finish(3, vt3)

        # ------------------------------------------------ F build + conv
        def build_F(h, eng):
            """F~ build for half h. eng = DMA engine for the gen_out loads."""
            pA, pB = 2 * h, 2 * h + 1
            fsb = fker_p.tile([128, NTAP, 2, GS], dt.bfloat16,
                              name=f"fsb{h}", tag=f"fsb{h}")
            # P^T loads (tap-independent): rows 0:64 = even group (cm), rows
            # 64:128 = odd group
            p128 = fload_p.tile([128, 2, GS], dt.bfloat16, name=f"pld{h}",
                                tag=f"pld{h}", bufs=1)
            eng.dma_start(
                out=p128[0:64, :, :],
                in_=gen_out[4 * h:4 * h + 3:2, NTAP, :].rearrange(
                    "g (cm co) -> cm g co", co=GS),
            )
            eng.dma_start(
                out=p128[64:128, :, :],
                in_=gen_out[4 * h + 1:4 * h + 4:2, NTAP, :].rearrange(
                    "g (cm co) -> cm g co", co=GS),
            )
            # round 1: taps 0-7 packed into one [128, 512] psum tile per geo
            ps07 = [psum_s.tile([128, 512], dt.float32, name=f"f07_{geo}", tag="s")
                    for geo in range(2)]
            wtiles = []
            for t in range(NTAP):
                w128 = fload_p.tile([128, 2, GS], dt.bfloat16, name="wld", tag="wld")
                eng.dma_start(
                    out=w128[0:64, :, :],
                    in_=gen_out[4 * h:4 * h + 3:2, t, :].rearrange(
                        "g (cm ci) -> cm g ci", ci=GS),
                )
                eng.dma_start(
                    out=w128[64:128, :, :],
                    in_=gen_out[4 * h + 1:4 * h + 4:2, t, :].rearrange(
                        "g (cm ci) -> cm g ci", ci=GS),
                )
                wtiles.append(w128)
                if t < 8:
                    for geo in range(2):
                        dst = ps07[geo][:, t * 64:(t + 1) * 64]
                        nc.tensor.matmul(out=dst[0:64, :], lhsT=w128[0:64, geo, :],
                                         rhs=p128[0:64, geo, :], start=True, stop=True,
                                         skip_group_check=True)
                        nc.tensor.matmul(out=dst[64:128, :], lhsT=w128[64:128, geo, :],
                                         rhs=p128[64:128, geo, :], start=True, stop=True,
                                         skip_group_check=True)
            # scale taps 0-7 by rstd (per input channel) + cast bf16 (VectorE)
            for geo in range(2):
                p_idx = pA if geo == 0 else pB
                nc.vector.tensor_scalar(
                    out=fsb[:, 0:8, geo, :],
                    in0=ps07[geo][:, :].rearrange("p (t c) -> p t c", t=8),
                    scalar1=rstd_sb[:, p_idx:p_idx + 1], scalar2=None, op0=ALU.mult)
            # round 2: tap 8 (cols 0:64) + B bias (col 64) per geo
            ps8 = [psum_s.tile([128, 512], dt.float32, name=f"f8_{geo}", tag="s")
                   for geo in range(2)]
            w8 = wtiles[8]
            for geo in range(2):
                nc.tensor.matmul(out=ps8[geo][0:64, 0:64], lhsT=w8[0:64, geo, :],
                                 rhs=p128[0:64, geo, :], start=True, stop=True,
                                 skip_group_check=True)
                nc.tensor.matmul(out=ps8[geo][64:128, 0:64], lhsT=w8[64:128, geo, :],
                                 rhs=p128[64:128, geo, :], start=True, stop=True,
                                 skip_group_check=True)
            for geo in range(2):
                p_idx = pA if geo == 0 else pB
                nc.vector.tensor_scalar(
                    out=fsb[:, 8, geo, :],
                    in0=ps8[geo][:, 0:64],
                    scalar1=rstd_sb[:, p_idx:p_idx + 1], scalar2=None, op0=ALU.mult)
            # tap-summed F~ (bf16, via f32 reduce then cast) for the B bias
            fsum_f = fker_p.tile([128, 2, GS], dt.float32, name=f"fsumf{h}",
                                 tag="fsumf", bufs=2)
            fsum = fker_p.tile([128, 2, GS], dt.bfloat16, name=f"fsum{h}",
                               tag="fsum", bufs=2)
            for geo in range(2):
                nc.vector.tensor_reduce(
                    out=fsum_f[:, geo, :],
                    in_=fsb[:, :, geo, :].rearrange("p t c -> p c t"),
                    axis=mybir.AxisListType.X, op=ALU.add,
                )
            nc.vector.tensor_scalar(
                out=fsum[:], in0=fsum_f[:],
                scalar1=1.0, scalar2=None, op0=ALU.mult)
            # -B accumulation: quadrant matmuls vs muneg; pair B swapped
            nc.tensor.matmul(out=ps8[0][0:64, 64:65], lhsT=fsum[0:64, 0, :],
                             rhs=muneg_sb[0:64, pA:pA + 1], start=True, stop=True,
                             skip_group_check=True)
            nc.tensor.matmul(out=ps8[0][64:128, 64:65], lhsT=fsum[64:128, 0, :],
                             rhs=muneg_sb[64:128, pA:pA + 1], start=True, stop=True,
                             skip_group_check=True)
            nc.tensor.matmul(out=ps8[1][64:128, 64:65], lhsT=fsum[0:64, 1, :],
                             rhs=muneg_sb[0:64, pB:pB + 1], start=True, stop=True,
                             skip_group_check=True)
            nc.tensor.matmul(out=ps8[1][0:64, 64:65], lhsT=fsum[64:128, 1, :],
                             rhs=muneg_sb[64:128, pB:pB + 1], start=True, stop=True,
                             skip_group_check=True)
            bias_A = const_p.tile([128, 1], dt.float32, name=f"biasA{h}", tag=f"bA{h}")
            bias_B = const_p.tile([128, 1], dt.float32, name=f"biasB{h}", tag=f"bB{h}")
            nc.vector.tensor_tensor(out=bias_A[:], in0=ps8[0][:, 64:65],
                                    in1=pwb_sb[:, pA:pA + 1], op=ALU.add)
            nc.vector.tensor_tensor(out=bias_B[:], in0=ps8[1][:, 64:65],
                                    in1=pwb_sb[:, pB:pB + 1], op=ALU.add)
            return fsb, bias_A, bias_B

        def conv_block(h, fsb, bias_A, bias_B, blk):
            pA, pB = 2 * h, 2 * h + 1
            imA = img[pA].rearrange("p (r c) -> p r c", c=PW)
            imB = img[pB].rearrange("p (r c) -> p r c", c=PW)
            y0 = blk * 4 * CONV_NT
            psA_ = [psum_c.tile([128, 512], dt.float32, name="cvA", tag="c")
                    for _ in range(CONV_NT)]
            psB_ = [psum_c.tile([128, 512], dt.float32, name="cvB", tag="c")
                    for _ in range(CONV_NT)]
            for t in range(NTAP):
                ky, kx = t // 3, t % 3
                st_ = (t == 0)
                sp_ = (t == NTAP - 1)
                for nt in range(CONV_NT):
                    y = y0 + 4 * nt
                    rA = imA[:, y + ky:y + ky + 4, kx:kx + 128]
                    rB = imB[:, y + ky:y + ky + 4, kx:kx + 128]
                    nc.tensor.matmul(out=psA_[nt][0:64, :], lhsT=fsb[0:64, t, 0, :],
                                     rhs=rA[0:64], start=st_, stop=sp_,
                                     skip_group_check=True)
                    nc.tensor.matmul(out=psA_[nt][64:128, :], lhsT=fsb[64:128, t, 0, :],
                                     rhs=rA[64:128], start=st_, stop=sp_,
                                     skip_group_check=True)
                    nc.tensor.matmul(out=psB_[nt][64:128, :], lhsT=fsb[0:64, t, 1, :],
                                     rhs=rB[0:64], start=st_, stop=sp_,
                                     skip_group_check=True)
                    nc.tensor.matmul(out=psB_[nt][0:64, :], lhsT=fsb[64:128, t, 1, :],
                                     rhs=rB[64:128], start=st_, stop=sp_,
                                     skip_group_check=True)
            # evacuate + bias -> bf16; ScalarE pair A; VectorE/GpSimd pair B
            stA = stage_p.tile([128, 512 * CONV_NT], dt.bfloat16, name="stA", tag="stg")
            stB = stage_p.tile([128, 512 * CONV_NT], dt.bfloat16, name="stB", tag="stg")
            evacB = nc.vector
            for nt in range(CONV_NT):
                nc.scalar.activation(
                    out=stA[:, nt * 512:(nt + 1) * 512], in_=psA_[nt][:],
                    func=AF.Identity, bias=bias_A[:, 0:1], scale=1.0,
                )
                evacB.tensor_scalar(
                    out=stB[:, nt * 512:(nt + 1) * 512], in0=psB_[nt][:],
                    scalar1=bias_B[:, 0:1], scalar2=None, op0=ALU.add,
                )
            px0 = y0 * 128
            nc.sync.dma_start(
                out=out_dev[pA * 128:(pA + 1) * 128, px0:px0 + 512 * CONV_NT],
                in_=stA[:],
            )
            nc.sync.dma_start(
                out=out_dev[pB * 128:(pB + 1) * 128, px0:px0 + 512 * CONV_NT],
                in_=stB[:],
            )

        NBLK = HW // (512 * CONV_NT)
        with tc.tile_pool(name="pssml", bufs=2, space="PSUM") as psum_s, \
             tc.tile_pool(name="psconv", bufs=6, space="PSUM") as psum_c:
            # pwb bias chain (device channel order) — runs during the AllToAll
            for m in range(4):
                pm = psum_c.tile([128, 512], dt.float32, name="pwbps", tag="c")
                for kt in range(4):
                    nc.tensor.matmul(
                        out=pm[:, 0:1],
                        lhsT=pwbt_sb[:, kt, m * 128:(m + 1) * 128],
                        rhs=sdnb[:, kt:kt + 1],
                        start=(kt == 0), stop=False,
                    )
                nc.tensor.matmul(
                    out=pm[:, 0:1],
                    lhsT=pwbb_sb[:1, m * 128:(m + 1) * 128],
                    rhs=ones[:1, 0:1],
                    start=False, stop=True,
                )
                nc.scalar.copy(out=pwb_sb[:, m:m + 1], in_=pm[:, 0:1])

            fsb0, bA0, bB0 = build_F(0, nc.sync)
            for blk in range(2):
                conv_block(0, fsb0, bA0, bB0, blk)
            fsb1, bA1, bB1 = build_F(1, nc.scalar)
            for blk in range(2, NBLK):
                conv_block(0, fsb0, bA0, bB0, blk)
            for blk in range(NBLK):
                conv_block(1, fsb1, bA1, bB1, blk)

    nc.compile()
    return nc


_NC_CACHE = {}


def kernel(**inputs) -> np.ndarray:
    from concourse.bass_utils import run_bass_kernel_spmd

    in_maps = _host_prep(**inputs)
    if "nc" not in _NC_CACHE:
        _NC_CACHE["nc"] = build_nc()
    nc = _NC_CACHE["nc"]
    res = run_bass_kernel_spmd(nc, in_maps, core_ids=list(range(N)))
    return _unshard(res.results)


if __name__ == "__main__":
    import jax

    import reference

    with jax.default_device(jax.devices("cpu")[0]):
        inputs = {k: np.asarray(v) for k, v in reference.setup_inputs().items()}
        expected = np.asarray(reference.reference(**inputs))
    actual = kernel(**inputs)
    err = np.sqrt(((actual - expected) ** 2).mean()) / np.sqrt((expected ** 2).mean())
    print("Relative error:", err)



# revision 20
# speedup vs baseline: 1.2208x; 1.2208x over previous
"""AdaConv (nn_AdaConv_46445776339355) — 8-core TRN2 Bass kernel, v3.

Strategy
--------
Data-parallel over batch N=8: core n owns sample n end-to-end for the heavy
instance-norm + grouped-conv work.  The kernel *generator* (dw_w is 256 MiB)
is tensor-parallel: core j holds an output-channel shard of dw_w / pwk_w,
computes the generated kernels for ALL samples on its shard, and an AllToAll
routes each sample's kernels to its owning core.

Algebraic fusions (all computed on device):
  * pointwise o depthwise = one fused per-group kernel  F_t = P @ W_t
  * instance norm folded into the fused kernels:
        y = sum_t F_t @ ((x-mu)/sigma)_pad = sum_t (F_t/sigma_ci) @ x_pad - B
    with B = sum_t (F_t/sigma) @ mu  (position independent, reflect-pad safe)
  * biases (dw_b, pwk_b, pwb_b) folded in via K=1 matmul rows.

The grouped conv (8 groups of 64->64 ch, 3x3) runs as 4 concurrent 64x64
matmuls in the 4 PE-array quadrants (tile_position packing), bf16, 9
shifted-AP taps accumulating in PSUM.

v3 changes (vs v2; trace-driven):
  * SPLIT generator + two half-size AllToAlls.  Each core's weight shard is
    re-cut into (X_j, Y_j): X_j = generated channels [2048*j, 2048*(j+1))
    (= m-half shards of conv groups 0-3), Y_j likewise for groups 4-7.
    A2A#1 fires as soon as the X half is generated and delivers groups 0-3
    complete, so F-build + conv half 0 start ~60us earlier; A2A#2 hides
    under conv half 0.  (v2's single A2A left an 86us Tensor-idle gap.)
  * DMA priority: the generator weight stream owns the queues first; images
    interleave after each half's weights instead of competing with them.
  * Compound conv matmuls: rhs [*, 8, 128] spans both 4-row output tiles of
    a block -> one LDWEIGHTS per (tap, quadrant) instead of two (v2 spent
    310us of LDWEIGHTS pipeline time; streams ran ~2.6/4 col/cycle).
    PSUM conv tiles are [128, 1024] (two banks), evacs 1024 px wide.
  * Engine-stream aware issue order (engine streams execute in order, and a
    semaphore wait blocks everything behind it): gpsimd owns gen staging /
    collective triggers / H1-odd chunk loads / F1 preloads; stats for pairs
    2,3 are interleaved into the conv-h0 loop behind their image loads.
"""

import sys
import numpy as np

sys.path.insert(0, "/opt/trn_rl_repo")

import ml_dtypes

BF16 = ml_dtypes.bfloat16

# ---------------- problem constants (hardcoded per the harness contract) ----
N = 8            # batch == number of cores
C = 512          # channels
H = W = 128
HW = H * W       # 16384
PW = W + 2       # 130 padded
PA = PW * PW     # 16900
SD = 512         # style dim
NG = 8           # groups
GS = 64          # group size (channels per group)
KDW = SD * 4     # 2048 contraction dim of the dw generator
NTAP = 9
EPS = 1e-5
VAR_CORR = float(HW) / float(HW - 1)  # ddof=1 correction

HSH = 2048                # half-shard width (gen output channels per A2A)
NKT = KDW // 128          # 16 contraction tiles for the dw generator
NSLOT = 2 * (4 + NKT)     # 40 weight slots of [128, HSH]
NCHUNK = NSLOT // 2       # 20 chunks of [128, 2, HSH]

# device output channel order: per pair of groups (2h, 2h+1) natural, the odd
# pairs (pB) have their two 64-blocks swapped (quadrant output packing).
TAU_BLOCK = [0, 1, 3, 2, 4, 5, 7, 6]  # true 64-block of device 64-block d


def _host_prep(style_encoding, predicted, dw_w, dw_b, pwk_w, pwk_b, pwb_w, pwb_b):
    """Pure data-movement / dtype-cast host prep. Returns per-core input maps."""
    f32 = np.float32
    se = np.asarray(style_encoding, f32)
    pred = np.asarray(predicted, f32)

    # --- patches for the dw generator conv: reflect pad 1, 2x2 windows s=2 ---
    sep = np.pad(se, ((0, 0), (0, 0), (1, 1), (1, 1)), mode="reflect")  # (8,512,6,6)
    blocks = sep.reshape(N, SD, 3, 2, 3, 2)  # [n,c,oy,ky,ox,kx]
    patches = np.ascontiguousarray(
        blocks.transpose(1, 3, 5, 0, 2, 4).reshape(KDW, N * NTAP)
    ).astype(BF16)  # [(c,ky,kx), (n,oy,ox)] = [2048, 72]
    patches_t = np.ascontiguousarray(
        patches.reshape(16, 128, N * NTAP).transpose(1, 0, 2).reshape(128, 16 * N * NTAP)
    )

    # --- dw generator weights, transposed: dwt_full[k, o] = dw_w[o, k] -------
    dwt_full = np.ascontiguousarray(dw_w.reshape(C * GS, KDW).T).astype(BF16)  # [2048, 32768]
    dwb_full = np.asarray(dw_b, f32).reshape(1, C * GS).astype(BF16)

    # --- pwk: permute columns to (g, cm, co2) so the gathered row IS P^T ---
    pwk_t = np.asarray(pwk_w, f32).reshape(NG, GS, GS, SD)  # [g, co2, cm, sd]
    pwkt_full = np.ascontiguousarray(
        pwk_t.transpose(3, 0, 2, 1).reshape(SD, C * GS)
    ).astype(BF16)  # [sd, (g, cm, co2)]
    pwkb_full = (
        np.asarray(pwk_b, f32).reshape(NG, GS, GS).transpose(0, 2, 1).reshape(1, C * GS)
    ).astype(BF16)

    # --- pwb: transposed, columns in DEVICE channel order tau ---
    tau_rows = np.concatenate([np.arange(GS) + t * GS for t in TAU_BLOCK])  # [512]
    pwbt = np.ascontiguousarray(np.asarray(pwb_w, f32)[tau_rows, :].T).astype(BF16)  # [sd, out_dev]
    pwbt_t = np.ascontiguousarray(
        pwbt.reshape(4, 128, C).transpose(1, 0, 2).reshape(128, 4 * C))
    pwbb = np.asarray(pwb_b, f32)[tau_rows].reshape(1, C).astype(BF16)

    # --- styleT for sd computation on device: pre-tiled [128, kt, (n, px)] f32 ---
    styleT = np.ascontiguousarray(se.transpose(1, 0, 2, 3).reshape(SD, N * 16)).astype(f32)
    styleT_t = np.ascontiguousarray(
        styleT.reshape(4, 128, N * 16).transpose(1, 0, 2).reshape(128, 4 * N * 16))

    in_maps = []
    for j in range(N):
        pp = np.pad(pred[j], ((0, 0), (1, 1), (1, 1)), mode="reflect").reshape(C, PA)
        sel = np.zeros((128, N), f32)
        sel[:, j] = 1.0
        X = slice(j * HSH, (j + 1) * HSH)                   # m-halves, groups 0-3
        Y = slice(16384 + j * HSH, 16384 + (j + 1) * HSH)   # m-halves, groups 4-7
        # unified weight stream: 40 slots of [128, 2048]:
        #   0-3 pwk-X kts, 4-19 dw-X kts, 20-23 pwk-Y kts, 24-39 dw-Y kts
        slots = np.concatenate([
            pwkt_full[:, X].reshape(4, 128, HSH),
            dwt_full[:, X].reshape(NKT, 128, HSH),
            pwkt_full[:, Y].reshape(4, 128, HSH),
            dwt_full[:, Y].reshape(NKT, 128, HSH),
        ], axis=0)  # [40, 128, 2048]
        w_all = np.ascontiguousarray(
            slots.transpose(1, 0, 2).reshape(128, NSLOT * HSH))
        dwb_j = np.ascontiguousarray(
            np.concatenate([dwb_full[:, X], dwb_full[:, Y]], axis=1))  # [1, 4096]
        pwkb_j = np.ascontiguousarray(
            np.concatenate([pwkb_full[:, X], pwkb_full[:, Y]], axis=1))
        in_maps.append(
            dict(
                pred_pad=np.ascontiguousarray(pp).astype(BF16),
                patches=patches_t,
                styleT=styleT_t,
                sel=sel,
                dwt=w_all,
                dwb=dwb_j,
                pwkb=pwkb_j,
                pwbt=pwbt_t,
                pwbb=pwbb,
            )
        )
    return in_maps


def _unshard(results):
    """results[j]['out'] is [512, 16384] bf16 in device channel order."""
    out = np.empty((N, C, H, W), np.float32)
    for j in range(N):
        dev = np.asarray(results[j]["out"]).astype(np.float32).reshape(C, H, W)
        for d, t in enumerate(TAU_BLOCK):
            out[j, t * GS:(t + 1) * GS] = dev[d * GS:(d + 1) * GS]
    return out


def build_nc(dbg=False):
    from concourse import bacc, mybir, tile
    from contextlib import ExitStack

    dt = mybir.dt
    AF = mybir.ActivationFunctionType
    ALU = mybir.AluOpType

    nc = bacc.Bacc(num_devices=N)

    if dbg:
        dbgf = nc.declare_dram_parameter("dbgf", [128, 32], dt.float32, isOutput=True)
        dbgb = nc.declare_dram_parameter("dbgb", [128, 3072], dt.bfloat16, isOutput=True)

    pred_pad = nc.declare_dram_parameter("pred_pad", [C, PA], dt.bfloat16, isOutput=False)
    patches = nc.declare_dram_parameter("patches", [128, 16 * N * NTAP], dt.bfloat16, isOutput=False)
    styleT = nc.declare_dram_parameter("styleT", [128, 4 * N * 16], dt.float32, isOutput=False)
    sel = nc.declare_dram_parameter("sel", [128, N], dt.float32, isOutput=False)
    dwt = nc.declare_dram_parameter("dwt", [128, NSLOT * HSH], dt.bfloat16, isOutput=False)
    dwb = nc.declare_dram_parameter("dwb", [1, 2 * HSH], dt.bfloat16, isOutput=False)
    pwkb = nc.declare_dram_parameter("pwkb", [1, 2 * HSH], dt.bfloat16, isOutput=False)
    pwbt = nc.declare_dram_parameter("pwbt", [128, 4 * C], dt.bfloat16, isOutput=False)
    pwbb = nc.declare_dram_parameter("pwbb", [1, C], dt.bfloat16, isOutput=False)
    out_dev = nc.declare_dram_parameter("out", [C, HW], dt.bfloat16, isOutput=True)

    replica = [list(range(N))]

    with tile.TileContext(nc) as tc, ExitStack() as ctx:
        dram = ctx.enter_context(tc.tile_pool(name="dram", bufs=1, space="DRAM"))
        gen_all1 = dram.tile([N, NTAP + 1, HSH], dt.bfloat16, tag="ga1")
        gen_out1 = dram.tile([N, NTAP + 1, HSH], dt.bfloat16, tag="go1")
        gen_all2 = dram.tile([N, NTAP + 1, HSH], dt.bfloat16, tag="ga2")
        gen_out2 = dram.tile([N, NTAP + 1, HSH], dt.bfloat16, tag="go2")

        const_p = ctx.enter_context(tc.tile_pool(name="const", bufs=1))
        pred_p = ctx.enter_context(tc.tile_pool(name="pred", bufs=1))
        stats_p = ctx.enter_context(tc.tile_pool(name="stats", bufs=2))
        fker_p = ctx.enter_context(tc.tile_pool(name="fker", bufs=1))
        fload_p = ctx.enter_context(tc.tile_pool(name="fload", bufs=3))
        # weight-stream + gen-staging pools are scoped: they close after the
        # generator phase, freeing SBUF for the evac staging pool below
        wstream_ctx = tc.tile_pool(name="wstream", bufs=3)
        wstream_p = wstream_ctx.__enter__()
        gstg_ctx = tc.tile_pool(name="gstg", bufs=2)
        gstg_p = gstg_ctx.__enter__()

        # ------------------------------------------------ tiny consts
        pt = const_p.tile([128, 16, N * NTAP], dt.bfloat16, tag="pt")
        nc.sync.dma_start(out=pt[:], in_=patches.rearrange("p (kt m) -> p kt m", kt=16))
        st = const_p.tile([128, 4, N * 16], dt.float32, tag="st")
        nc.sync.dma_start(out=st[:], in_=styleT.rearrange("p (kt m) -> p kt m", kt=4))
        sel_sb = const_p.tile([128, N], dt.float32, tag="sel")
        nc.sync.dma_start(out=sel_sb[:], in_=sel[:, :])
        pwbt_sb = const_p.tile([128, 4, C], dt.bfloat16, tag="pwbt")
        nc.scalar.dma_start(out=pwbt_sb[:], in_=pwbt.rearrange("p (kt m) -> p kt m", kt=4))
        pwbb_sb = const_p.tile([1, C], dt.bfloat16, tag="pwbb")
        nc.scalar.dma_start(out=pwbb_sb[:], in_=pwbb[:, :])
        # bias rows on partition 0; per-half tiles, single-buffered (H1's
        # load rides gpsimd after the A2A#1 trigger, reusing H0's buffer)
        pkb_t, dwb_t = {}, {}

        def load_biases(hh):
            pkb_t[hh] = gstg_p.tile([1, HSH], dt.bfloat16, name=f"pkb{hh}",
                                    tag="pkb", bufs=1)
            nc.gpsimd.dma_start(out=pkb_t[hh][:],
                                in_=pwkb[0:1, hh * HSH:(hh + 1) * HSH])
            dwb_t[hh] = gstg_p.tile([1, HSH], dt.bfloat16, name=f"dwb{hh}",
                                    tag="dwbb", bufs=1)
            nc.gpsimd.dma_start(out=dwb_t[hh][:],
                                in_=dwb[0:1, hh * HSH:(hh + 1) * HSH])

        load_biases(0)

        # ------------------------------------------------ big input streams
        # Weight stream: 20 chunks [128, 2, 2048] (8 KiB/partition).  H0
        # chunks: even on sync, odd on scalar; then img p0 (sync) / p1, p3
        # (scalar).  H1 chunks: even on sync (after img p0), odd on gpsimd
        # (issued after A2A#1 trigger); then img p2 (sync).
        w_v = dwt.rearrange("p (s m) -> p s m", s=NSLOT)
        w_ch = [None] * NCHUNK
        img = [pred_p.tile([128, PA], dt.bfloat16, name=f"img{p}", tag=f"img{p}")
               for p in range(4)]

        def load_chunk(c, eng):
            wtile = wstream_p.tile([128, 2, HSH], dt.bfloat16, name=f"wt{c}", tag="w")
            eng.dma_start(out=wtile[:], in_=w_v[:, 2 * c:2 * c + 2, :])
            w_ch[c] = wtile

        for c in range(0, 10, 2):
            load_chunk(c, nc.sync)
            load_chunk(c + 1, nc.scalar)
        nc.sync.dma_start(out=img[0][:], in_=pred_pad[0:128, :])
        nc.scalar.dma_start(out=img[1][:], in_=pred_pad[128:256, :])
        nc.scalar.dma_start(out=img[3][:], in_=pred_pad[384:512, :])

        ones = const_p.tile([1, 128], dt.bfloat16, tag="ones")
        nc.vector.memset(ones[:], 1.0)

        # ------------------------------------------------ sd = mean(style, px)
        sdf = const_p.tile([128, 4, N], dt.float32, tag="sdf")
        sdb = const_p.tile([128, 4, N], dt.bfloat16, tag="sdb")
        sdnb = const_p.tile([128, 4], dt.bfloat16, tag="sdnb")  # own-sample column
        tmp8 = stats_p.tile([128, N], dt.float32, tag="tmp8")
        sdn_f = const_p.tile([128, 4], dt.float32, tag="sdnf")
        for kt in range(4):
            nc.vector.tensor_reduce(
                out=sdf[:, kt, :],
                in_=st[:, kt, :].rearrange("p (n x) -> p n x", x=16),
                axis=mybir.AxisListType.X,
                op=ALU.add,
            )
            nc.vector.tensor_scalar(
                out=sdb[:, kt, :], in0=sdf[:, kt, :], scalar1=1.0 / 16.0,
                scalar2=None, op0=ALU.mult,
            )
            nc.vector.tensor_tensor(
                out=tmp8[:], in0=sdf[:, kt, :], in1=sel_sb[:], op=ALU.mult
            )
            nc.vector.tensor_reduce(
                out=sdn_f[:, kt:kt + 1], in_=tmp8[:], axis=mybir.AxisListType.X, op=ALU.add
            )
        nc.vector.tensor_scalar(
            out=sdnb[:], in0=sdn_f[:], scalar1=1.0 / 16.0, scalar2=None, op0=ALU.mult
        )

        # ------------------------------------------------ instance-norm stats
        rstd_sb = const_p.tile([128, 4], dt.float32, tag="rstd")
        muneg_sb = const_p.tile([128, 4], dt.bfloat16, tag="muneg")
        sq_dummy = stats_p.tile([128, 8 * 128], dt.float8e4, tag="sqdmy", bufs=1)
        pwb_sb = const_p.tile([128, 4], dt.float32, tag="pwbv")
        sacc = [None] * 4

        def stats_sq(p, j0, j1):
            """ScalarE: sacc[p][:, j] = sum over chunk j of x^2 (Square+accum)."""
            view = img[p].rearrange("p (r c) -> p r c", c=PW)
            if sacc[p] is None:
                sacc[p] = stats_p.tile([128, 16], dt.float32, name=f"sacc{p}",
                                       tag=f"sacc{p}")
            dview = sq_dummy.rearrange("p (r c) -> p r c", c=128)
            for j in range(j0, j1):
                xs = view[:, 1 + 8 * j:1 + 8 * (j + 1), 1:129]
                nc.scalar.activation(
                    out=dview[:], in_=xs, func=AF.Square,
                    accum_out=sacc[p][:, j:j + 1],
                )

        def stats_sum(p):
            """VectorE: full-image sum + combines -> muneg, var+eps."""
            view = img[p].rearrange("p (r c) -> p r c", c=PW)
            ssum = stats_p.tile([128, 1], dt.float32, name="ssum", tag="ssum")
            nc.vector.tensor_reduce(
                out=ssum[:], in_=view[:, 1:129, 1:129],
                axis=mybir.AxisListType.XY, op=ALU.add
            )
            ssq = stats_p.tile([128, 1], dt.float32, name="ssq", tag="ssq")
            nc.vector.tensor_reduce(
                out=ssq[:], in_=sacc[p][:], axis=mybir.AxisListType.X, op=ALU.add
            )
            mu = stats_p.tile([128, 1], dt.float32, name="mu", tag="mu")
            nc.vector.tensor_scalar(
                out=mu[:], in0=ssum[:], scalar1=1.0 / HW, scalar2=None, op0=ALU.mult
            )
            nc.vector.tensor_scalar(
                out=muneg_sb[:, p:p + 1], in0=mu[:], scalar1=-1.0,
                scalar2=None, op0=ALU.mult,
            )
            ex2 = stats_p.tile([128, 1], dt.float32, name="ex2", tag="ex2")
            nc.vector.tensor_scalar(
                out=ex2[:], in0=ssq[:], scalar1=1.0 / HW, scalar2=None, op0=ALU.mult
            )
            mu2 = stats_p.tile([128, 1], dt.float32, name="mu2", tag="mu2")
            nc.vector.tensor_tensor(out=mu2[:], in0=mu[:], in1=mu[:], op=ALU.mult)
            varp = stats_p.tile([128, 1], dt.float32, name="varp", tag="varp")
            nc.vector.tensor_tensor(out=varp[:], in0=ex2[:], in1=mu2[:], op=ALU.subtract)
            vtmp = stats_p.tile([128, 1], dt.float32, name="vtmp", tag=f"sm2_{p}")
            nc.vector.tensor_scalar(
                out=vtmp[:], in0=varp[:], scalar1=VAR_CORR, scalar2=EPS,
                op0=ALU.mult, op1=ALU.add,
            )
            return vtmp

        def stats_finish(p, vtmp):
            stdt = stats_p.tile([128, 1], dt.float32, name="stdt", tag=f"sm3_{p}")
            nc.scalar.sqrt(stdt[:], vtmp[:])
            nc.vector.reciprocal(out=rstd_sb[:, p:p + 1], in_=stdt[:])

        # ------------------------------------------------ generator phase (PE)
        with tc.tile_pool(name="psgen", bufs=8, space="PSUM") as psum_g:
            # pwb bias chain (device channel order) first: only needs consts
            for m in range(4):
                pm = psum_g.tile([128, 512], dt.float32, name="pwbps", tag="g")
                for kt in range(4):
                    nc.tensor.matmul(
                        out=pm[:, 0:1],
                        lhsT=pwbt_sb[:, kt, m * 128:(m + 1) * 128],
                        rhs=sdnb[:, kt:kt + 1],
                        start=(kt == 0), stop=False,
                    )
                nc.tensor.matmul(
                    out=pm[:, 0:1],
                    lhsT=pwbb_sb[:1, m * 128:(m + 1) * 128],
                    rhs=ones[:1, 0:1],
                    start=False, stop=True,
                )
                nc.scalar.copy(out=pwb_sb[:, m:m + 1], in_=pm[:, 0:1])

            def gen_half(hh, gen_allX):
                """Generate half-shard hh (0: X / groups 0-3, 1: Y / 4-7)."""
                base = hh * (NCHUNK // 2)  # first chunk of this half
                # pwk generator: 4 kts (chunks base, base+1), 4 banks of 512
                ps_pk = [psum_g.tile([128, 512], dt.float32, name=f"pk{hh}{b}",
                                     tag="g") for b in range(4)]
                for kt in range(4):
                    ch = w_ch[base + kt // 2]
                    for b in range(4):
                        nc.tensor.matmul(
                            out=ps_pk[b][:N, :],
                            lhsT=sdb[:, kt, :],
                            rhs=ch[:, kt % 2, b * 512:(b + 1) * 512],
                            start=(kt == 0), stop=False,
                        )
                gsP = gstg_p.tile([N, HSH], dt.bfloat16, name=f"gsP{hh}",
                                  tag="gsP", bufs=1)
                for b in range(4):
                    nc.tensor.matmul(
                        out=ps_pk[b][:N, :],
                        lhsT=ones[:1, :N],
                        rhs=pkb_t[hh][0:1, b * 512:(b + 1) * 512],
                        start=False, stop=True,
                    )
                    nc.vector.tensor_scalar(out=gsP[:, b * 512:(b + 1) * 512],
                                            in0=ps_pk[b][:N, :],
                                            scalar1=1.0, scalar2=None, op0=ALU.mult)
                # dw generator: 16 kts (chunks base+2 .. base+9)
                ps_dw = [psum_g.tile([128, 512], dt.float32, name=f"dw{hh}{b}",
                                     tag="g") for b in range(4)]
                for kt in range(NKT):
                    ch = w_ch[base + 2 + kt // 2]
                    for b in range(4):
                        nc.tensor.matmul(
                            out=ps_dw[b][:N * NTAP, :],
                            lhsT=pt[:, kt, :],
                            rhs=ch[:, kt % 2, b * 512:(b + 1) * 512],
                            start=(kt == 0), stop=False,
                        )
                gsD = gstg_p.tile([N * NTAP, HSH], dt.bfloat16, name=f"gsD{hh}",
                                  tag="gsD", bufs=1)
                for b in range(4):
                    nc.tensor.matmul(
                        out=ps_dw[b][:N * NTAP, :],
                        lhsT=ones[:1, :N * NTAP],
                        rhs=dwb_t[hh][0:1, b * 512:(b + 1) * 512],
                        start=False, stop=True,
                    )
                    nc.vector.tensor_scalar(out=gsD[:, b * 512:(b + 1) * 512],
                                            in0=ps_dw[b][:N * NTAP, :],
                                            scalar1=1.0, scalar2=None, op0=ALU.mult)
                # stage to DRAM (gpsimd queue; tiny).  NOTE: gsD is passed
                # flat — rearranging an SBUF partition dim mis-addresses the
                # read; the DMA balancer splits the DRAM side instead.
                nc.gpsimd.dma_start(out=gen_allX[:, 0:NTAP, :], in_=gsD[:, :])
                nc.gpsimd.dma_start(out=gen_allX[:, NTAP, :], in_=gsP[:, :])
                if dbg and hh == 0:
                    nc.scalar.dma_start(out=dbgb[0:72, 0:64], in_=gsD[:, 0:64])
                    nc.scalar.dma_start(out=dbgb[0:8, 64:128], in_=gsP[:, 0:64])

            # ---- half 0: generate, exchange
            gen_half(0, gen_all1)
            nc.gpsimd.collective_compute(
                "AllToAll",
                ALU.bypass,
                replica_groups=replica,
                ins=[gen_all1[:, :, :].opt()],
                outs=[gen_out1[:, :, :].opt()],
            )
            # H1 biases + odd chunks ride the gpsimd queue after the A2A#1
            # trigger; H1-even chunks + img p2 on sync (after img p0).
            # Chunk tiles MUST be requested in consumption order (c10..c19)
            # so the wstream buffer rotation cannot cross-block.
            load_biases(1)
            for c in range(10, 20):
                load_chunk(c, nc.sync if c % 2 == 0 else nc.gpsimd)
            nc.sync.dma_start(out=img[2][:], in_=pred_pad[256:384, :])

            # stats for pairs 0,1 while A2A#1 is in flight
            stats_sq(0, 0, 16)
            stats_sq(1, 0, 16)
            vt0 = stats_sum(0)
            stats_finish(0, vt0)
            vt1 = stats_sum(1)
            stats_finish(1, vt1)

            # ---- half 1: generate, exchange
            gen_half(1, gen_all2)
            nc.gpsimd.collective_compute(
                "AllToAll",
                ALU.bypass,
                replica_groups=replica,
                ins=[gen_all2[:, :, :].opt()],
                outs=[gen_out2[:, :, :].opt()],
            )

        # ------------------------------------------------ F build + conv
        def load_F(h, eng):
            """DMA the gen_out rows for half h into SBUF gather tiles.

            gen_outX rows j = (local group gg = j//2, m-half = j%2); row
            content [2048] = (m32, q|co).  Partition target (par, mh, m32)
            with par = group parity within pair, geo = pair; gg = 2*geo+par.
            """
            go = gen_out1 if h == 0 else gen_out2

            def gather(tile_, t):
                for par in range(2):
                    for mh in range(2):
                        p0 = par * 64 + mh * 32
                        eng.dma_start(
                            out=tile_[p0:p0 + 32, :, :],
                            in_=go[2 * par + mh:N:4, t, :].rearrange(
                                "geo (m co) -> m geo co", co=GS),
                        )

            p128 = fload_p.tile([128, 2, GS], dt.bfloat16, name=f"pld{h}",
                                tag=f"pld{h}", bufs=1)
            gather(p128, NTAP)
            wtiles = []
            for t in range(NTAP):
                w128 = fload_p.tile([128, 2, GS], dt.bfloat16, name=f"wld{h}_{t}",
                                    tag=f"wld{h}_{t}", bufs=1)
                gather(w128, t)
                wtiles.append(w128)
            return p128, wtiles

        def build_F(h, p128, wtiles):
            """F~ build for half h from preloaded gather tiles."""
            pA, pB = 2 * h, 2 * h + 1
            fsb = fker_p.tile([128, NTAP, 2, GS], dt.bfloat16,
                              name=f"fsb{h}", tag=f"fsb{h}")
            # round 1: taps 0-7 packed into one [128, 512] psum tile per geo
            ps07 = [psum_s.tile([128, 512], dt.float32, name=f"f07_{geo}", tag="s")
                    for geo in range(2)]
            for t in range(8):
                w128 = wtiles[t]
                for geo in range(2):
                    dst = ps07[geo][:, t * 64:(t + 1) * 64]
                    nc.tensor.matmul(out=dst[0:64, :], lhsT=w128[0:64, geo, :],
                                     rhs=p128[0:64, geo, :], start=True, stop=True,
                                     skip_group_check=True)
                    nc.tensor.matmul(out=dst[64:128, :], lhsT=w128[64:128, geo, :],
                                     rhs=p128[64:128, geo, :], start=True, stop=True,
                                     skip_group_check=True)
            # scale taps 0-7 by rstd (per input channel) + cast bf16 (VectorE)
            for geo in range(2):
                p_idx = pA if geo == 0 else pB
                nc.vector.tensor_scalar(
                    out=fsb[:, 0:8, geo, :],
                    in0=ps07[geo][:, :].rearrange("p (t c) -> p t c", t=8),
                    scalar1=rstd_sb[:, p_idx:p_idx + 1], scalar2=None, op0=ALU.mult)
            # round 2: tap 8 (cols 0:64) + B bias (col 64) per geo
            ps8 = [psum_s.tile([128, 512], dt.float32, name=f"f8_{geo}", tag="s")
                   for geo in range(2)]
            w8 = wtiles[8]
            for geo in range(2):
                nc.tensor.matmul(out=ps8[geo][0:64, 0:64], lhsT=w8[0:64, geo, :],
                                 rhs=p128[0:64, geo, :], start=True, stop=True,
                                 skip_group_check=True)
                nc.tensor.matmul(out=ps8[geo][64:128, 0:64], lhsT=w8[64:128, geo, :],
                                 rhs=p128[64:128, geo, :], start=True, stop=True,
                                 skip_group_check=True)
            for geo in range(2):
                p_idx = pA if geo == 0 else pB
                nc.vector.tensor_scalar(
                    out=fsb[:, 8, geo, :],
                    in0=ps8[geo][:, 0:64],
                    scalar1=rstd_sb[:, p_idx:p_idx + 1], scalar2=None, op0=ALU.mult)
            # tap-summed F~ (bf16, via f32 reduce then cast) for the B bias
            fsum_f = fker_p.tile([128, 2, GS], dt.float32, name=f"fsumf{h}",
                                 tag="fsumf", bufs=2)
            fsum = fker_p.tile([128, 2, GS], dt.bfloat16, name=f"fsum{h}",
                               tag="fsum", bufs=2)
            for geo in range(2):
                nc.vector.tensor_reduce(
                    out=fsum_f[:, geo, :],
                    in_=fsb[:, :, geo, :].rearrange("p t c -> p c t"),
                    axis=mybir.AxisListType.X, op=ALU.add,
                )
            nc.vector.tensor_scalar(
                out=fsum[:], in0=fsum_f[:],
                scalar1=1.0, scalar2=None, op0=ALU.mult)
            # -B accumulation: quadrant matmuls vs muneg; pair B swapped
            nc.tensor.matmul(out=ps8[0][0:64, 64:65], lhsT=fsum[0:64, 0, :],
                             rhs=muneg_sb[0:64, pA:pA + 1], start=True, stop=True,
                             skip_group_check=True)
            nc.tensor.matmul(out=ps8[0][64:128, 64:65], lhsT=fsum[64:128, 0, :],
                             rhs=muneg_sb[64:128, pA:pA + 1], start=True, stop=True,
                             skip_group_check=True)
            nc.tensor.matmul(out=ps8[1][64:128, 64:65], lhsT=fsum[0:64, 1, :],
                             rhs=muneg_sb[0:64, pB:pB + 1], start=True, stop=True,
                             skip_group_check=True)
            nc.tensor.matmul(out=ps8[1][0:64, 64:65], lhsT=fsum[64:128, 1, :],
                             rhs=muneg_sb[64:128, pB:pB + 1], start=True, stop=True,
                             skip_group_check=True)
            bias_A = const_p.tile([128, 1], dt.float32, name=f"biasA{h}", tag=f"bA{h}")
            bias_B = const_p.tile([128, 1], dt.float32, name=f"biasB{h}", tag=f"bB{h}")
            nc.vector.tensor_tensor(out=bias_A[:], in0=ps8[0][:, 64:65],
                                    in1=pwb_sb[:, pA:pA + 1], op=ALU.add)
            nc.vector.tensor_tensor(out=bias_B[:], in0=ps8[1][:, 64:65],
                                    in1=pwb_sb[:, pB:pB + 1], op=ALU.add)
            return fsb, bias_A, bias_B

        def conv_block(h, fsb, bias_A, bias_B, blk):
            """One block = 8 output rows (1024 px) = 2 psum banks per pair.
            Matmuls sharing a lhsT are issued back-to-back so the NEFF
            codegen can elide the repeated LDWEIGHTS."""
            pA, pB = 2 * h, 2 * h + 1
            imA = img[pA].rearrange("p (r c) -> p r c", c=PW)
            imB = img[pB].rearrange("p (r c) -> p r c", c=PW)
            y0 = blk * 8
            psA_ = [psum_c.tile([128, 512], dt.float32, name="cvA", tag="c")
                    for _ in range(2)]
            psB_ = [psum_c.tile([128, 512], dt.float32, name="cvB", tag="c")
                    for _ in range(2)]
            for t in range(NTAP):
                ky, kx = t // 3, t % 3
                st_ = (t == 0)
                sp_ = (t == NTAP - 1)
                quads = (
                    (psA_, imA, slice(0, 64), 0, slice(0, 64)),
                    (psA_, imA, slice(64, 128), 0, slice(64, 128)),
                    (psB_, imB, slice(0, 64), 1, slice(64, 128)),
                    (psB_, imB, slice(64, 128), 1, slice(0, 64)),
                )
                for ps_, im, rsl, geo, osl in quads:
                    for nt in range(2):
                        y = y0 + 4 * nt
                        r = im[:, y + ky:y + ky + 4, kx:kx + 128]
                        nc.tensor.matmul(out=ps_[nt][osl, :],
                                         lhsT=fsb[rsl, t, geo, :],
                                         rhs=r[rsl], start=st_, stop=sp_,
                                         skip_group_check=True)
            # evacuate + bias -> bf16; ScalarE pair A; VectorE pair B
            stA = stage_p.tile([128, 1024], dt.bfloat16, name="stA", tag="stg")
            stB = stage_p.tile([128, 1024], dt.bfloat16, name="stB", tag="stg")
            for nt in range(2):
                nc.scalar.activation(
                    out=stA[:, nt * 512:(nt + 1) * 512], in_=psA_[nt][:],
                    func=AF.Identity, bias=bias_A[:, 0:1], scale=1.0,
                )
                nc.vector.tensor_scalar(
                    out=stB[:, nt * 512:(nt + 1) * 512], in0=psB_[nt][:],
                    scalar1=bias_B[:, 0:1], scalar2=None, op0=ALU.add,
                )
            px0 = y0 * 128
            nc.sync.dma_start(
                out=out_dev[pA * 128:(pA + 1) * 128, px0:px0 + 1024],
                in_=stA[:],
            )
            nc.scalar.dma_start(
                out=out_dev[pB * 128:(pB + 1) * 128, px0:px0 + 1024],
                in_=stB[:],
            )

        NBLK = HW // 1024  # 16
        with tc.tile_pool(name="pssml", bufs=2, space="PSUM") as psum_s, \
             tc.tile_pool(name="psconv", bufs=6, space="PSUM") as psum_c:
            # F1 gather loads ride the gpsimd queue (idle after trig#2); they
            # wait on A2A#2 completion without blocking any other stream.
            p128_1, wtiles_1 = load_F(1, nc.gpsimd)
            # F0 loads on sync (after img p2; wait on A2A#1)
            p128_0, wtiles_0 = load_F(0, nc.sync)
            fsb0, bA0, bB0 = build_F(0, p128_0, wtiles_0)

            # weight stream fully consumed once gen ran — the tiles are dead;
            # close pools so the allocator can hand the space to stage_p
            gstg_ctx.__exit__(None, None, None)
            wstream_ctx.__exit__(None, None, None)
            stage_p = ctx.enter_context(tc.tile_pool(name="stage", bufs=3))

            if dbg:
                nc.scalar.dma_start(out=dbgf[:, 0:4], in_=rstd_sb[:, :])
                nc.scalar.dma_start(out=dbgf[:, 8:12], in_=pwb_sb[:, :])
                nc.scalar.dma_start(out=dbgf[:, 12:13], in_=bA0[:, :])
                nc.scalar.dma_start(out=dbgf[:, 13:14], in_=bB0[:, :])
                nc.scalar.dma_start(out=dbgb[:, 128:132], in_=muneg_sb[:, :])
                nc.scalar.dma_start(
                    out=dbgb[:, 256:384],
                    in_=p128_0.rearrange("p a b -> p (a b)"))
                nc.scalar.dma_start(
                    out=dbgb[:, 384:512],
                    in_=p128_1.rearrange("p a b -> p (a b)"))
                nc.scalar.dma_start(
                    out=dbgb[:, 512:1664],
                    in_=fsb0.rearrange("p a b c -> p (a b c)"))
                nc.scalar.dma_start(
                    out=dbgb[:, 1664:1792],
                    in_=wtiles_0[0].rearrange("p a b -> p (a b)"))
                for tt in range(4, 8):
                    nc.scalar.dma_start(
                        out=dbgb[:, 1792 + (tt - 4) * 128:1792 + (tt - 3) * 128],
                        in_=wtiles_0[tt].rearrange("p a b -> p (a b)"))
            fsb1 = bA1 = bB1 = None
            for blk in range(NBLK):
                conv_block(0, fsb0, bA0, bB0, blk)
                # interleave pair-2/3 stats into the scalar/vector streams
                if blk < 4:
                    stats_sq(2, 4 * blk, 4 * blk + 4)
                elif blk == 4:
                    vt2 = stats_sum(2)
                    stats_finish(2, vt2)
                elif blk < 9:
                    stats_sq(3, 4 * (blk - 5), 4 * (blk - 5) + 4)
                elif blk == 9:
                    vt3 = stats_sum(3)
                    stats_finish(3, vt3)
                elif blk == 12:
                    fsb1, bA1, bB1 = build_F(1, p128_1, wtiles_1)
            for blk in range(NBLK):
                conv_block(1, fsb1, bA1, bB1, blk)

    nc.compile()
    return nc


_NC_CACHE = {}


def kernel(**inputs) -> np.ndarray:
    from concourse.bass_utils import run_bass_kernel_spmd

    in_maps = _host_prep(**inputs)
    if "nc" not in _NC_CACHE:
        _NC_CACHE["nc"] = build_nc()
    nc = _NC_CACHE["nc"]
    res = run_bass_kernel_spmd(nc, in_maps, core_ids=list(range(N)))
    return _unshard(res.results)


if __name__ == "__main__":
    import jax

    import reference

    with jax.default_device(jax.devices("cpu")[0]):
        inputs = {k: np.asarray(v) for k, v in reference.setup_inputs().items()}
        expected = np.asarray(reference.reference(**inputs))
    actual = kernel(**inputs)
    err = np.sqrt(((actual - expected) ** 2).mean()) / np.sqrt((expected ** 2).mean())
    print("Relative error:", err)
